# revision 45
# baseline (speedup 1.0000x reference)
"""Causal multi-head attention (B=4, T=2048, D=1024, H=16, HD=64) on 8
Trainium2 NeuronCores.

Sharding: data-parallel over batch (4) x tensor-parallel over heads (2
groups of 8). Each core runs the same Bass program on its own input
slices; the host sums the two tensor-parallel partial projections per
batch and adds b_proj.

Per-core dataflow (feature-major, no on-chip transposes), software-
pipelined across chunks of 512 query tokens so ScalarE exp overlaps the
QKV / proj matmuls:

  chunk c:  QKV(c) on PE  ->  attn units (4 head-pairs) for chunk c
            (S row-tiled 64-deep matmul pairs, exp on ScalarE, AV in
            bf16)  ->  proj(c), while QKV(c+1) fills PE gaps.

All weights persist in SBUF (loaded once). Q/K/S stay float32r
(1 cycle/row at N>=256); V, exp(S), O and w_proj are bf16 (1 cycle/row
at every N, fast weight load on proj).
"""

import numpy as np
import ml_dtypes

import concourse.bass as bass
import concourse.bacc as bacc
import concourse.mybir as mybir
import concourse.tile as tile
from concourse.bass_utils import run_bass_kernel_spmd

F32 = mybir.dt.float32
F32R = mybir.dt.float32r
BF16 = mybir.dt.bfloat16
AF = mybir.ActivationFunctionType

B, T, D = 4, 2048, 1024
H, HD = 16, 64
NH = 8          # heads per core
DL = NH * HD    # 512 local qkv feature dim
PAIRS = NH // 2
CH = T // 512   # 4 chunks of 512 tokens
KT = T // 128   # 16 tk blocks / token tiles
VW = 66         # V columns per head incl. ones column + pad (4B alignment)


def build(nc: bass.Bass):
    xT = nc.declare_dram_parameter("xT", [D, T], F32R, isOutput=False)
    wq = nc.declare_dram_parameter("wq", [D, DL], F32R, isOutput=False)
    wk = nc.declare_dram_parameter("wk", [D, DL], F32R, isOutput=False)
    wv = nc.declare_dram_parameter("wv", [D, DL], F32R, isOutput=False)
    bq = nc.declare_dram_parameter("bq", [DL], F32, isOutput=False)
    bk = nc.declare_dram_parameter("bk", [DL], F32, isOutput=False)
    bv = nc.declare_dram_parameter("bv", [DL], F32, isOutput=False)
    wp = nc.declare_dram_parameter("wp", [DL, D], BF16, isOutput=False)
    ones8 = nc.declare_dram_parameter("ones8", [128, 16], BF16,
                                      isOutput=False)
    yT = nc.declare_dram_parameter("yT", [D, T], F32, isOutput=True)

    with tile.TileContext(nc) as tc:
        with (
            tc.tile_pool(name="persist", bufs=1) as persist,
            tc.tile_pool(name="qpool", bufs=8) as qpool,
            tc.tile_pool(name="opool", bufs=8) as opool,
            tc.tile_pool(name="xpool", bufs=16) as xpool,
            tc.tile_pool(name="etpool", bufs=6) as etpool,
            tc.tile_pool(name="opsb", bufs=4) as opsb,
            tc.tile_pool(name="accps", bufs=2, space="PSUM") as accps,
            tc.tile_pool(name="strips", bufs=2, space="PSUM") as strips,
            tc.tile_pool(name="avps", bufs=2, space="PSUM") as avps,
            tc.tile_pool(name="dram", bufs=4, space="DRAM") as dram,
        ):
            # -------- persistent tiles (weights, K^T, V) --------
            kT = [persist.tile([128, T], F32R, tag=f"kt{p}", name=f"kt{p}")
                  for p in range(PAIRS)]
            v_sb = [persist.tile([128, NH * VW], BF16, tag=f"v{i}",
                                 name=f"v{i}") for i in range(KT)]
            wq_sb = [persist.tile([128, DL], F32R, tag=f"wq{k}",
                                  name=f"wq{k}") for k in range(8)]
            wk_sb = [persist.tile([128, DL], F32R, tag=f"wk{k}",
                                  name=f"wk{k}") for k in range(8)]
            wv_sb = [persist.tile([128, DL], F32R, tag=f"wv{k}",
                                  name=f"wv{k}") for k in range(8)]
            wp_sb = [persist.tile([128, D], BF16, tag=f"wp{k}",
                                  name=f"wp{k}") for k in range(4)]
            bq_sb = persist.tile([128, 4], F32, tag="bq", name="bq_sb")
            bk_sb = persist.tile([128, 4], F32, tag="bk", name="bk_sb")
            bv_bc = persist.tile([128, DL], F32, tag="bv", name="bv_bc")
            def load_xt(c):
                cs = slice(512 * c, 512 * c + 512)
                xt = []
                for k in range(8):
                    t_ = xpool.tile([128, 512], F32R, tag="xt", name="xt")
                    nc.sync.dma_start(
                        out=t_, in_=xT[128 * k : 128 * k + 128, cs]
                    )
                    xt.append(t_)
                return xt

            # V-projection weights + x chunk 0 first, interleaved in the
            # order the first matmul group consumes them.  Everything else
            # loads behind them.
            xt0 = []
            for k in range(8):
                nc.sync.dma_start(out=wv_sb[k],
                                  in_=wv[128 * k : 128 * k + 128, :])
                t_ = xpool.tile([128, 512], F32R, tag="xt", name="xt")
                nc.sync.dma_start(
                    out=t_, in_=xT[128 * k : 128 * k + 128, 0:512]
                )
                xt0.append(t_)
            nc.sync.dma_start(
                out=bv_bc,
                in_=bass.AP(tensor=bv[:].tensor, offset=0,
                            ap=[[0, 128], [1, DL]]),
            )
            # ones (+pad) columns of every V tile, written once
            for tt in range(KT):
                vv = v_sb[tt].rearrange("p (h c) -> p h c", c=VW)
                nc.sync.dma_start(
                    out=vv[:, :, HD : HD + 2],
                    in_=ones8[:, 0:16].rearrange("p (h c) -> p h c", c=2),
                )
            for k in range(8):
                nc.sync.dma_start(out=wq_sb[k],
                                  in_=wq[128 * k : 128 * k + 128, :])
                nc.sync.dma_start(out=wk_sb[k],
                                  in_=wk[128 * k : 128 * k + 128, :])
            for k in range(4):
                nc.sync.dma_start(out=wp_sb[k],
                                  in_=wp[128 * k : 128 * k + 128, :])
            nc.sync.dma_start(
                out=bq_sb, in_=bq[:].rearrange("(a p) -> p a", p=128)
            )
            nc.sync.dma_start(
                out=bk_sb, in_=bk[:].rearrange("(a p) -> p a", p=128)
            )

            def qkv_v(c, xt):
                """V projection for token chunk c -> v_sb[4c..4c+4]."""
                for t4 in range(4):
                    tt = 4 * c + t4
                    acc = accps.tile([128, 512], F32, tag="acc", name="acc")
                    for k in range(8):
                        nc.tensor.matmul(
                            acc,
                            xt[k][:, 128 * t4 : 128 * t4 + 128],
                            wv_sb[k],
                            start=(k == 0), stop=(k == 7),
                        )
                    for h in range(NH):
                        nc.vector.tensor_add(
                            v_sb[tt][:, VW * h : VW * h + HD],
                            acc[:, HD * h : HD * h + HD],
                            bv_bc[:, HD * h : HD * h + HD],
                        )

            def qkv_qk(c, qts, xt):
                """Q^T / K^T projections for token chunk c."""
                cs = slice(512 * c, 512 * c + 512)
                for w_sb, b_sb, dsts in (
                    (wq_sb, bq_sb, [(qts[n], slice(0, 512)) for n in range(4)]),
                    (wk_sb, bk_sb, [(kT[n], cs) for n in range(4)]),
                ):
                    for n in range(4):
                        acc = accps.tile([128, 512], F32, tag="acc",
                                         name="acc")
                        for k in range(8):
                            nc.tensor.matmul(
                                acc,
                                w_sb[k][:, 128 * n : 128 * n + 128],
                                xt[k],
                                start=(k == 0), stop=(k == 7),
                            )
                        dst, dcs = dsts[n]
                        nc.vector.tensor_scalar_add(
                            out=dst[:, dcs],
                            in0=acc,
                            scalar1=b_sb[:, n : n + 1],
                        )

            def attn_unit(p, c, qt):
                """Attention for head-pair p, query chunk c. Returns the
                normalized O^T tile [128, 512] (bf16)."""
                kt = kT[p]
                ha, hb = 2 * p, 2 * p + 1
                nb = 4 * (c + 1)
                av_a = avps.tile([VW, 512], F32, tag="av", name="av_a")
                av_b = avps.tile([VW, 512], F32, tag="av", name="av_b")
                for b in range(nb):
                    diag = (b // 4 == c)
                    off = 128 * (b - 4 * c) if diag else 0
                    bs = slice(128 * b, 128 * b + 128)
                    strip = strips.tile([128, 1024], F32, tag="strip",
                                        name="strip")
                    et = etpool.tile([128, 1024], BF16, tag="et", name="et")
                    nc.tensor.matmul(
                        strip[:, off:512],
                        kt[0:64, bs],
                        qt[0:64, off:512],
                        start=True, stop=True,
                    )
                    nc.tensor.matmul(
                        strip[:, 512 + off : 1024],
                        kt[64:128, bs],
                        qt[64:128, off:512],
                        start=True, stop=True,
                    )
                    if off == 0:
                        nc.scalar.activation(
                            et[:, 0:1024], strip[:, 0:1024], AF.Exp
                        )
                    else:
                        # one instr over both heads' valid regions:
                        # cols [off,512) and [512+off,1024)
                        w_ = 512 - off
                        src_ap = bass.AP(
                            tensor=strip.tensor,
                            offset=strip.offset + off,
                            ap=[list(strip.ap[0]), [512, 2], [1, w_]],
                        )
                        dst_ap = bass.AP(
                            tensor=et.tensor,
                            offset=et.offset + off,
                            ap=[list(et.ap[0]), [512, 2], [1, w_]],
                        )
                        nc.scalar.activation(dst_ap, src_ap, AF.Exp)
                    if diag:
                        for bcol in (off, 512 + off):
                            nc.gpsimd.affine_select(
                                out=et[:, bcol : bcol + 128],
                                in_=et[:, bcol : bcol + 128],
                                compare_op=mybir.AluOpType.is_ge,
                                fill=0.0,
                                base=0,
                                pattern=[[1, 128]],
                                channel_multiplier=-1,
                            )
                    nc.tensor.matmul(
                        av_a[:, off:512],
                        v_sb[b][:, VW * ha : VW * ha + VW],
                        et[:, off:512],
                        start=(b == 0), stop=(b == nb - 1),
                    )
                    nc.tensor.matmul(
                        av_b[:, off:512],
                        v_sb[b][:, VW * hb : VW * hb + VW],
                        et[:, 512 + off : 1024],
                        start=(b == 0), stop=(b == nb - 1),
                    )
                osb_t = opool.tile([128, 512], BF16, tag="osb", name="osb")
                for h, av in ((0, av_a), (1, av_b)):
                    # evict PSUM early (frees the bank for the next unit)
                    oraw = opsb.tile([64, 512], F32, tag="oraw", name="oraw")
                    nc.vector.tensor_copy(oraw, av[0:64, :])
                    den = opsb.tile([1, 512], F32, tag="den", name="den",
                                    bufs=2)
                    nc.vector.tensor_copy(den, av[64:65, :])
                    rec = opsb.tile([1, 512], F32, tag="rec", name="rec",
                                    bufs=2)
                    scr = opsb.tile([1, 512], F32, tag="scr", name="scr",
                                    bufs=2)
                    nc.vector.reciprocal_approx_accurate(
                        rec, den, scratch=scr
                    )
                    # broadcast 1/den across 64 partitions via a DRAM bounce
                    rd = dram.tile([1, 512], F32, tag="rd", name="rd")
                    nc.sync.dma_start(out=rd, in_=rec)
                    bc = opsb.tile([64, 512], F32, tag="bc", name="bc",
                                   bufs=2)
                    nc.sync.dma_start(
                        out=bc,
                        in_=bass.AP(tensor=rd.tensor, offset=rd.offset,
                                    ap=[[0, 64]] + list(rd.ap[1:])),
                    )
                    nc.vector.tensor_mul(
                        osb_t[64 * h : 64 * h + 64, :],
                        oraw[0:64, :],
                        bc,
                    )
                return osb_t

            def proj_chunk(c, osbs):
                for n in range(8):
                    acc = accps.tile([128, 512], F32, tag="acc", name="acc")
                    for k in range(4):
                        nc.tensor.matmul(
                            acc,
                            wp_sb[k][:, 128 * n : 128 * n + 128],
                            osbs[k],
                            start=(k == 0), stop=(k == 3),
                        )
                    yt = opsb.tile([128, 512], F32, tag="yt", name="yt",
                                   bufs=4)
                    nc.vector.tensor_copy(yt, acc)
                    nc.sync.dma_start(
                        out=yT[128 * n : 128 * n + 128,
                               512 * c : 512 * c + 512],
                        in_=yt,
                    )

            qts = [qpool.tile([128, 512], F32R, tag="qt", name="qt")
                   for _ in range(PAIRS)]
            qkv_v(0, xt0)
            qkv_qk(0, qts, xt0)
            for c in range(CH):
                if c + 1 < CH:
                    xt_next = load_xt(c + 1)
                    qts2 = [qpool.tile([128, 512], F32R, tag="qt", name="qt")
                            for _ in range(PAIRS)]
                with nc.named_scope(f"attn{c}"):
                    osbs = [attn_unit(0, c, qts[0]),
                            attn_unit(1, c, qts[1])]
                if c + 1 < CH:
                    with nc.named_scope(f"qkv{c + 1}"):
                        qkv_v(c + 1, xt_next)
                with nc.named_scope(f"attn{c}b"):
                    osbs += [attn_unit(2, c, qts[2]),
                             attn_unit(3, c, qts[3])]
                if c + 1 < CH:
                    with nc.named_scope(f"qkv{c + 1}"):
                        qkv_qk(c + 1, qts2, xt_next)
                    qts = qts2
                with nc.named_scope(f"proj{c}"):
                    proj_chunk(c, osbs)
    return nc


_prog = None


def _get_program():
    global _prog
    if _prog is None:
        _prog = build(bacc.Bacc(None))
        _prog.finalize()
    return _prog


def make_in_maps(x, w_qkv, b_qkv, w_proj):
    x = np.ascontiguousarray(np.asarray(x, np.float32))
    w_qkv = np.asarray(w_qkv, np.float32)
    b_qkv = np.asarray(b_qkv, np.float32)
    w_proj = np.asarray(w_proj, np.float32)
    in_maps = []
    for core in range(8):
        b, g = divmod(core, 2)
        gs = slice(DL * g, DL * g + DL)
        gk = slice(D + DL * g, D + DL * g + DL)
        gv = slice(2 * D + DL * g, 2 * D + DL * g + DL)
        in_maps.append({
            "xT": np.ascontiguousarray(x[b].T),
            "wq": np.ascontiguousarray(w_qkv[:, gs]) * np.float32(0.125),
            "wk": np.ascontiguousarray(w_qkv[:, gk]),
            "wv": np.ascontiguousarray(w_qkv[:, gv]),
            "bq": np.ascontiguousarray(b_qkv[gs]) * np.float32(0.125),
            "bk": np.ascontiguousarray(b_qkv[gk]),
            "bv": np.ascontiguousarray(b_qkv[gv]),
            "wp": np.ascontiguousarray(
                w_proj[DL * g : DL * g + DL, :]).astype(ml_dtypes.bfloat16),
            "ones8": np.ones((128, 16), ml_dtypes.bfloat16),
        })
    return in_maps


def combine_outputs(results, b_proj):
    b_proj = np.asarray(b_proj, np.float32)
    y = np.empty((B, T, D), np.float32)
    for b in range(B):
        yt = results[2 * b]["yT"] + results[2 * b + 1]["yT"]
        y[b] = yt.T + b_proj
    return y


def kernel(x, w_qkv, b_qkv, w_proj, b_proj, **run_kwargs):
    in_maps = make_in_maps(x, w_qkv, b_qkv, w_proj)
    r = run_bass_kernel_spmd(_get_program(), in_maps,
                             core_ids=list(range(8)), **run_kwargs)
    out = combine_outputs(r.results, b_proj)
    kernel.last_result = r
    return out


# revision 46
# speedup vs baseline: 1.0290x; 1.0290x over previous
"""Causal multi-head attention (B=4, T=2048, D=1024, H=16, HD=64) on 8
Trainium2 NeuronCores.

Sharding: data-parallel over batch (4) x tensor-parallel over heads (2
groups of 8). Each core runs the same Bass program on its own input
slices; the host sums the two tensor-parallel partial projections per
batch and adds b_proj.

Per-core dataflow (feature-major, no on-chip transposes), software-
pipelined across chunks of 512 query tokens so ScalarE exp overlaps the
QKV / proj matmuls:

  chunk c:  QKV(c) on PE  ->  attn units (4 head-pairs) for chunk c
            (S row-tiled 64-deep matmul pairs, exp on ScalarE, AV in
            bf16)  ->  proj(c), while QKV(c+1) fills PE gaps.

All weights persist in SBUF (loaded once). Q/K/S stay float32r
(1 cycle/row at N>=256); V, exp(S), O and w_proj are bf16 (1 cycle/row
at every N, fast weight load on proj).
"""

import numpy as np
import ml_dtypes

import concourse.bass as bass
import concourse.bacc as bacc
import concourse.mybir as mybir
import concourse.tile as tile
from concourse.bass_utils import run_bass_kernel_spmd

F32 = mybir.dt.float32
F32R = mybir.dt.float32r
BF16 = mybir.dt.bfloat16
AF = mybir.ActivationFunctionType

B, T, D = 4, 2048, 1024
H, HD = 16, 64
NH = 8          # heads per core
DL = NH * HD    # 512 local qkv feature dim
PAIRS = NH // 2
CH = T // 512   # 4 chunks of 512 tokens
KT = T // 128   # 16 tk blocks / token tiles
VW = 66         # V columns per head incl. ones column + pad (4B alignment)


def build(nc: bass.Bass):
    xT = nc.declare_dram_parameter("xT", [D, T], F32R, isOutput=False)
    wq = nc.declare_dram_parameter("wq", [D, DL], F32R, isOutput=False)
    wk = nc.declare_dram_parameter("wk", [D, DL], F32R, isOutput=False)
    wv = nc.declare_dram_parameter("wv", [D, DL], F32R, isOutput=False)
    bq = nc.declare_dram_parameter("bq", [DL], F32, isOutput=False)
    bk = nc.declare_dram_parameter("bk", [DL], F32, isOutput=False)
    bv = nc.declare_dram_parameter("bv", [DL], F32, isOutput=False)
    wp = nc.declare_dram_parameter("wp", [DL, D], BF16, isOutput=False)
    ones8 = nc.declare_dram_parameter("ones8", [128, 16], BF16,
                                      isOutput=False)
    yT = nc.declare_dram_parameter("yT", [D, T], F32, isOutput=True)

    with tile.TileContext(nc) as tc:
        with (
            tc.tile_pool(name="persist", bufs=1) as persist,
            tc.tile_pool(name="qpool", bufs=8) as qpool,
            tc.tile_pool(name="opool", bufs=8) as opool,
            tc.tile_pool(name="xpool", bufs=16) as xpool,
            tc.tile_pool(name="etpool", bufs=4) as etpool,
            tc.tile_pool(name="opsb", bufs=4) as opsb,
            tc.tile_pool(name="accps", bufs=2, space="PSUM") as accps,
            tc.tile_pool(name="strips", bufs=2, space="PSUM") as strips,
            tc.tile_pool(name="avps", bufs=2, space="PSUM") as avps,
            tc.tile_pool(name="dram", bufs=4, space="DRAM") as dram,
        ):
            # -------- persistent tiles (weights, K^T, V) --------
            kT = [persist.tile([128, T], F32R, tag=f"kt{p}", name=f"kt{p}")
                  for p in range(PAIRS)]
            v_sb = [persist.tile([128, NH * VW], BF16, tag=f"v{i}",
                                 name=f"v{i}") for i in range(KT)]
            wq_sb = [persist.tile([128, DL], F32R, tag=f"wq{k}",
                                  name=f"wq{k}") for k in range(8)]
            wk_sb = [persist.tile([128, DL], F32R, tag=f"wk{k}",
                                  name=f"wk{k}") for k in range(8)]
            wv_sb = [persist.tile([128, DL], F32R, tag=f"wv{k}",
                                  name=f"wv{k}") for k in range(8)]
            wp_sb = [persist.tile([128, D], BF16, tag=f"wp{k}",
                                  name=f"wp{k}") for k in range(4)]
            bq_sb = persist.tile([128, 4], F32, tag="bq", name="bq_sb")
            bk_sb = persist.tile([128, 4], F32, tag="bk", name="bk_sb")
            bv_bc = persist.tile([128, DL], F32, tag="bv", name="bv_bc")
            def load_xt(c):
                cs = slice(512 * c, 512 * c + 512)
                xt = []
                for k in range(8):
                    t_ = xpool.tile([128, 512], F32R, tag="xt", name="xt")
                    nc.sync.dma_start(
                        out=t_, in_=xT[128 * k : 128 * k + 128, cs]
                    )
                    xt.append(t_)
                return xt

            # V-projection weights + x chunk 0 first: they gate the first
            # matmuls.  Everything else loads behind them.
            for k in range(8):
                nc.sync.dma_start(out=wv_sb[k],
                                  in_=wv[128 * k : 128 * k + 128, :])
            xt0 = load_xt(0)
            nc.sync.dma_start(
                out=bv_bc,
                in_=bass.AP(tensor=bv[:].tensor, offset=0,
                            ap=[[0, 128], [1, DL]]),
            )
            # ones (+pad) columns of every V tile, written once
            for tt in range(KT):
                vv = v_sb[tt].rearrange("p (h c) -> p h c", c=VW)
                nc.sync.dma_start(
                    out=vv[:, :, HD : HD + 2],
                    in_=ones8[:, 0:16].rearrange("p (h c) -> p h c", c=2),
                )
            for k in range(8):
                nc.sync.dma_start(out=wq_sb[k],
                                  in_=wq[128 * k : 128 * k + 128, :])
                nc.sync.dma_start(out=wk_sb[k],
                                  in_=wk[128 * k : 128 * k + 128, :])
            for k in range(4):
                nc.sync.dma_start(out=wp_sb[k],
                                  in_=wp[128 * k : 128 * k + 128, :])
            nc.sync.dma_start(
                out=bq_sb, in_=bq[:].rearrange("(a p) -> p a", p=128)
            )
            nc.sync.dma_start(
                out=bk_sb, in_=bk[:].rearrange("(a p) -> p a", p=128)
            )

            def qkv_v(c, xt):
                """V projection for token chunk c -> v_sb[4c..4c+4]."""
                for t4 in range(4):
                    tt = 4 * c + t4
                    acc = accps.tile([128, 512], F32, tag="acc", name="acc")
                    for k in range(8):
                        nc.tensor.matmul(
                            acc,
                            xt[k][:, 128 * t4 : 128 * t4 + 128],
                            wv_sb[k],
                            start=(k == 0), stop=(k == 7),
                        )
                    for h in range(NH):
                        nc.vector.tensor_add(
                            v_sb[tt][:, VW * h : VW * h + HD],
                            acc[:, HD * h : HD * h + HD],
                            bv_bc[:, HD * h : HD * h + HD],
                        )

            def qkv_qk(c, qts, xt):
                """Q^T / K^T projections for token chunk c."""
                cs = slice(512 * c, 512 * c + 512)
                for w_sb, b_sb, dsts in (
                    (wq_sb, bq_sb, [(qts[n], slice(0, 512)) for n in range(4)]),
                    (wk_sb, bk_sb, [(kT[n], cs) for n in range(4)]),
                ):
                    for n in range(4):
                        acc = accps.tile([128, 512], F32, tag="acc",
                                         name="acc")
                        for k in range(8):
                            nc.tensor.matmul(
                                acc,
                                w_sb[k][:, 128 * n : 128 * n + 128],
                                xt[k],
                                start=(k == 0), stop=(k == 7),
                            )
                        dst, dcs = dsts[n]
                        nc.vector.tensor_scalar_add(
                            out=dst[:, dcs],
                            in0=acc,
                            scalar1=b_sb[:, n : n + 1],
                        )

            def attn_unit(p, c, qt):
                """Attention for head-pair p, query chunk c. Returns the
                normalized O^T tile [128, 512] (bf16)."""
                kt = kT[p]
                ha, hb = 2 * p, 2 * p + 1
                nb = 4 * (c + 1)
                av_a = avps.tile([VW, 512], F32, tag="av", name="av_a")
                av_b = avps.tile([VW, 512], F32, tag="av", name="av_b")
                for b in range(nb):
                    diag = (b // 4 == c)
                    off = 128 * (b - 4 * c) if diag else 0
                    bs = slice(128 * b, 128 * b + 128)
                    strip = strips.tile([128, 1024], F32, tag="strip",
                                        name="strip")
                    et = etpool.tile([128, 1024], BF16, tag="et", name="et")
                    nc.tensor.matmul(
                        strip[:, off:512],
                        kt[0:64, bs],
                        qt[0:64, off:512],
                        start=True, stop=True,
                    )
                    nc.tensor.matmul(
                        strip[:, 512 + off : 1024],
                        kt[64:128, bs],
                        qt[64:128, off:512],
                        start=True, stop=True,
                    )
                    if off == 0:
                        nc.scalar.activation(
                            et[:, 0:1024], strip[:, 0:1024], AF.Exp
                        )
                    else:
                        # one instr over both heads' valid regions:
                        # cols [off,512) and [512+off,1024)
                        w_ = 512 - off
                        src_ap = bass.AP(
                            tensor=strip.tensor,
                            offset=strip.offset + off,
                            ap=[list(strip.ap[0]), [512, 2], [1, w_]],
                        )
                        dst_ap = bass.AP(
                            tensor=et.tensor,
                            offset=et.offset + off,
                            ap=[list(et.ap[0]), [512, 2], [1, w_]],
                        )
                        nc.scalar.activation(dst_ap, src_ap, AF.Exp)
                    if diag:
                        for bcol in (off, 512 + off):
                            nc.gpsimd.affine_select(
                                out=et[:, bcol : bcol + 128],
                                in_=et[:, bcol : bcol + 128],
                                compare_op=mybir.AluOpType.is_ge,
                                fill=0.0,
                                base=0,
                                pattern=[[1, 128]],
                                channel_multiplier=-1,
                            )
                    nc.tensor.matmul(
                        av_a[:, off:512],
                        v_sb[b][:, VW * ha : VW * ha + VW],
                        et[:, off:512],
                        start=(b == 0), stop=(b == nb - 1),
                    )
                    nc.tensor.matmul(
                        av_b[:, off:512],
                        v_sb[b][:, VW * hb : VW * hb + VW],
                        et[:, 512 + off : 1024],
                        start=(b == 0), stop=(b == nb - 1),
                    )
                osb_t = opool.tile([128, 512], BF16, tag="osb", name="osb")
                for h, av in ((0, av_a), (1, av_b)):
                    # evict PSUM early (frees the bank for the next unit)
                    oraw = opsb.tile([64, 512], F32, tag="oraw", name="oraw")
                    nc.vector.tensor_copy(oraw, av[0:64, :])
                    den = opsb.tile([1, 512], F32, tag="den", name="den",
                                    bufs=2)
                    nc.vector.tensor_copy(den, av[64:65, :])
                    rec = opsb.tile([1, 512], F32, tag="rec", name="rec",
                                    bufs=2)
                    scr = opsb.tile([1, 512], F32, tag="scr", name="scr",
                                    bufs=2)
                    nc.vector.reciprocal_approx_accurate(
                        rec, den, scratch=scr
                    )
                    # broadcast 1/den across 64 partitions via a DRAM bounce
                    rd = dram.tile([1, 512], F32, tag="rd", name="rd")
                    nc.sync.dma_start(out=rd, in_=rec)
                    bc = opsb.tile([64, 512], F32, tag="bc", name="bc",
                                   bufs=2)
                    nc.sync.dma_start(
                        out=bc,
                        in_=bass.AP(tensor=rd.tensor, offset=rd.offset,
                                    ap=[[0, 64]] + list(rd.ap[1:])),
                    )
                    nc.vector.tensor_mul(
                        osb_t[64 * h : 64 * h + 64, :],
                        oraw[0:64, :],
                        bc,
                    )
                return osb_t

            def proj_chunk(c, osbs):
                for n in range(8):
                    acc = accps.tile([128, 512], F32, tag="acc", name="acc")
                    for k in range(4):
                        nc.tensor.matmul(
                            acc,
                            wp_sb[k][:, 128 * n : 128 * n + 128],
                            osbs[k],
                            start=(k == 0), stop=(k == 3),
                        )
                    yt = opsb.tile([128, 512], F32, tag="yt", name="yt",
                                   bufs=4)
                    nc.vector.tensor_copy(yt, acc)
                    nc.sync.dma_start(
                        out=yT[128 * n : 128 * n + 128,
                               512 * c : 512 * c + 512],
                        in_=yt,
                    )

            qts = [qpool.tile([128, 512], F32R, tag="qt", name="qt")
                   for _ in range(PAIRS)]
            qkv_v(0, xt0)
            qkv_qk(0, qts, xt0)
            for c in range(CH):
                if c + 1 < CH:
                    xt_next = load_xt(c + 1)
                    qts2 = [qpool.tile([128, 512], F32R, tag="qt", name="qt")
                            for _ in range(PAIRS)]
                with nc.named_scope(f"attn{c}"):
                    osbs = [attn_unit(0, c, qts[0]),
                            attn_unit(1, c, qts[1])]
                if c + 1 < CH:
                    with nc.named_scope(f"qkv{c + 1}"):
                        qkv_v(c + 1, xt_next)
                with nc.named_scope(f"attn{c}b"):
                    osbs += [attn_unit(2, c, qts[2]),
                             attn_unit(3, c, qts[3])]
                if c + 1 < CH:
                    with nc.named_scope(f"qkv{c + 1}"):
                        qkv_qk(c + 1, qts2, xt_next)
                    qts = qts2
                with nc.named_scope(f"proj{c}"):
                    proj_chunk(c, osbs)
    return nc


_prog = None


def _get_program():
    global _prog
    if _prog is None:
        _prog = build(bacc.Bacc(None))
        _prog.finalize()
    return _prog


def make_in_maps(x, w_qkv, b_qkv, w_proj):
    x = np.ascontiguousarray(np.asarray(x, np.float32))
    w_qkv = np.asarray(w_qkv, np.float32)
    b_qkv = np.asarray(b_qkv, np.float32)
    w_proj = np.asarray(w_proj, np.float32)
    in_maps = []
    for core in range(8):
        b, g = divmod(core, 2)
        gs = slice(DL * g, DL * g + DL)
        gk = slice(D + DL * g, D + DL * g + DL)
        gv = slice(2 * D + DL * g, 2 * D + DL * g + DL)
        in_maps.append({
            "xT": np.ascontiguousarray(x[b].T),
            "wq": np.ascontiguousarray(w_qkv[:, gs]) * np.float32(0.125),
            "wk": np.ascontiguousarray(w_qkv[:, gk]),
            "wv": np.ascontiguousarray(w_qkv[:, gv]),
            "bq": np.ascontiguousarray(b_qkv[gs]) * np.float32(0.125),
            "bk": np.ascontiguousarray(b_qkv[gk]),
            "bv": np.ascontiguousarray(b_qkv[gv]),
            "wp": np.ascontiguousarray(
                w_proj[DL * g : DL * g + DL, :]).astype(ml_dtypes.bfloat16),
            "ones8": np.ones((128, 16), ml_dtypes.bfloat16),
        })
    return in_maps


def combine_outputs(results, b_proj):
    b_proj = np.asarray(b_proj, np.float32)
    y = np.empty((B, T, D), np.float32)
    for b in range(B):
        yt = results[2 * b]["yT"] + results[2 * b + 1]["yT"]
        y[b] = yt.T + b_proj
    return y


def kernel(x, w_qkv, b_qkv, w_proj, b_proj, **run_kwargs):
    in_maps = make_in_maps(x, w_qkv, b_qkv, w_proj)
    r = run_bass_kernel_spmd(_get_program(), in_maps,
                             core_ids=list(range(8)), **run_kwargs)
    out = combine_outputs(r.results, b_proj)
    kernel.last_result = r
    return out


# revision 48
# speedup vs baseline: 1.0757x; 1.0455x over previous
"""Causal multi-head attention (B=4, T=2048, D=1024, H=16, HD=64) on 8
Trainium2 NeuronCores.

Sharding: data-parallel over batch (4) x tensor-parallel over heads (2
groups of 8). Each core runs the same Bass program on its own input
slices; the host sums the two tensor-parallel partial projections per
batch and adds b_proj.

Per-core dataflow (feature-major, no on-chip transposes), software-
pipelined across chunks of 512 query tokens so ScalarE exp overlaps the
QKV / proj matmuls:

  chunk c:  QKV(c) on PE  ->  attn units (4 head-pairs) for chunk c
            (S row-tiled 64-deep matmul pairs, exp on ScalarE, AV in
            bf16)  ->  proj(c), while QKV(c+1) fills PE gaps.

All weights persist in SBUF (loaded once). Q/K/S stay float32r
(1 cycle/row at N>=256); V, exp(S), O and w_proj are bf16 (1 cycle/row
at every N, fast weight load on proj).
"""

import numpy as np
import ml_dtypes

import concourse.bass as bass
import concourse.bacc as bacc
import concourse.mybir as mybir
import concourse.tile as tile
from concourse.bass_utils import run_bass_kernel_spmd

F32 = mybir.dt.float32
F32R = mybir.dt.float32r
BF16 = mybir.dt.bfloat16
AF = mybir.ActivationFunctionType

B, T, D = 4, 2048, 1024
H, HD = 16, 64
NH = 8          # heads per core
DL = NH * HD    # 512 local qkv feature dim
PAIRS = NH // 2
CH = T // 512   # 4 chunks of 512 tokens
KT = T // 128   # 16 tk blocks / token tiles
VW = 66         # V columns per head incl. ones column + pad (4B alignment)


def build(nc: bass.Bass):
    xT = nc.declare_dram_parameter("xT", [D, T], F32R, isOutput=False)
    wq = nc.declare_dram_parameter("wq", [D, DL], F32R, isOutput=False)
    wk = nc.declare_dram_parameter("wk", [D, DL], F32R, isOutput=False)
    wv = nc.declare_dram_parameter("wv", [D, DL], F32R, isOutput=False)
    bq = nc.declare_dram_parameter("bq", [DL], F32, isOutput=False)
    bk = nc.declare_dram_parameter("bk", [DL], F32, isOutput=False)
    bv = nc.declare_dram_parameter("bv", [DL], F32, isOutput=False)
    wp = nc.declare_dram_parameter("wp", [DL, D], BF16, isOutput=False)
    ones8 = nc.declare_dram_parameter("ones8", [128, 16], BF16,
                                      isOutput=False)
    yT = nc.declare_dram_parameter("yT", [D, T], F32, isOutput=True)

    with tile.TileContext(nc) as tc:
        with (
            tc.tile_pool(name="persist", bufs=1) as persist,
            tc.tile_pool(name="qpool", bufs=8) as qpool,
            tc.tile_pool(name="opool", bufs=8) as opool,
            tc.tile_pool(name="xpool", bufs=16) as xpool,
            tc.tile_pool(name="etpool", bufs=4) as etpool,
            tc.tile_pool(name="opsb", bufs=4) as opsb,
            tc.tile_pool(name="accps", bufs=2, space="PSUM") as accps,
            tc.tile_pool(name="strips", bufs=2, space="PSUM") as strips,
            tc.tile_pool(name="avps", bufs=2, space="PSUM") as avps,
            tc.tile_pool(name="dram", bufs=4, space="DRAM") as dram,
        ):
            # -------- persistent tiles (weights, K^T, V) --------
            kT = [persist.tile([128, T], BF16, tag=f"kt{p}", name=f"kt{p}")
                  for p in range(PAIRS)]
            v_sb = [persist.tile([128, NH * VW], BF16, tag=f"v{i}",
                                 name=f"v{i}") for i in range(KT)]
            wq_sb = [persist.tile([128, DL], F32R, tag=f"wq{k}",
                                  name=f"wq{k}") for k in range(8)]
            wk_sb = [persist.tile([128, DL], F32R, tag=f"wk{k}",
                                  name=f"wk{k}") for k in range(8)]
            wv_sb = [persist.tile([128, DL], F32R, tag=f"wv{k}",
                                  name=f"wv{k}") for k in range(8)]
            wp_sb = [persist.tile([128, D], BF16, tag=f"wp{k}",
                                  name=f"wp{k}") for k in range(4)]
            bq_sb = persist.tile([128, 4], F32, tag="bq", name="bq_sb")
            bk_sb = persist.tile([128, 4], F32, tag="bk", name="bk_sb")
            bv_bc = persist.tile([128, DL], F32, tag="bv", name="bv_bc")
            def load_xt(c):
                cs = slice(512 * c, 512 * c + 512)
                xt = []
                for k in range(8):
                    t_ = xpool.tile([128, 512], F32R, tag="xt", name="xt")
                    nc.sync.dma_start(
                        out=t_, in_=xT[128 * k : 128 * k + 128, cs]
                    )
                    xt.append(t_)
                return xt

            # V-projection weights + x chunk 0 first: they gate the first
            # matmuls.  Everything else loads behind them.
            for k in range(8):
                nc.sync.dma_start(out=wv_sb[k],
                                  in_=wv[128 * k : 128 * k + 128, :])
            xt0 = load_xt(0)
            nc.sync.dma_start(
                out=bv_bc,
                in_=bass.AP(tensor=bv[:].tensor, offset=0,
                            ap=[[0, 128], [1, DL]]),
            )
            # ones (+pad) columns of every V tile, written once
            for tt in range(KT):
                vv = v_sb[tt].rearrange("p (h c) -> p h c", c=VW)
                nc.sync.dma_start(
                    out=vv[:, :, HD : HD + 2],
                    in_=ones8[:, 0:16].rearrange("p (h c) -> p h c", c=2),
                )
            for k in range(8):
                nc.sync.dma_start(out=wq_sb[k],
                                  in_=wq[128 * k : 128 * k + 128, :])
                nc.sync.dma_start(out=wk_sb[k],
                                  in_=wk[128 * k : 128 * k + 128, :])
            for k in range(4):
                nc.sync.dma_start(out=wp_sb[k],
                                  in_=wp[128 * k : 128 * k + 128, :])
            nc.sync.dma_start(
                out=bq_sb, in_=bq[:].rearrange("(a p) -> p a", p=128)
            )
            nc.sync.dma_start(
                out=bk_sb, in_=bk[:].rearrange("(a p) -> p a", p=128)
            )

            def qkv_v(c, xt):
                """V projection for token chunk c -> v_sb[4c..4c+4]."""
                for t4 in range(4):
                    tt = 4 * c + t4
                    acc = accps.tile([128, 512], F32, tag="acc", name="acc")
                    for k in range(8):
                        nc.tensor.matmul(
                            acc,
                            xt[k][:, 128 * t4 : 128 * t4 + 128],
                            wv_sb[k],
                            start=(k == 0), stop=(k == 7),
                        )
                    for h in range(NH):
                        nc.vector.tensor_add(
                            v_sb[tt][:, VW * h : VW * h + HD],
                            acc[:, HD * h : HD * h + HD],
                            bv_bc[:, HD * h : HD * h + HD],
                        )

            def qkv_qk(c, qts, xt):
                """Q^T / K^T projections for token chunk c."""
                cs = slice(512 * c, 512 * c + 512)
                for w_sb, b_sb, dsts in (
                    (wq_sb, bq_sb, [(qts[n], slice(0, 512)) for n in range(4)]),
                    (wk_sb, bk_sb, [(kT[n], cs) for n in range(4)]),
                ):
                    for n in range(4):
                        acc = accps.tile([128, 512], F32, tag="acc",
                                         name="acc")
                        for k in range(8):
                            nc.tensor.matmul(
                                acc,
                                w_sb[k][:, 128 * n : 128 * n + 128],
                                xt[k],
                                start=(k == 0), stop=(k == 7),
                            )
                        dst, dcs = dsts[n]
                        nc.vector.tensor_scalar_add(
                            out=dst[:, dcs],
                            in0=acc,
                            scalar1=b_sb[:, n : n + 1],
                        )

            def attn_unit(p, c, qt):
                """Attention for head-pair p, query chunk c. Returns the
                normalized O^T tile [128, 512] (bf16)."""
                kt = kT[p]
                ha, hb = 2 * p, 2 * p + 1
                nb = 4 * (c + 1)
                av_a = avps.tile([VW, 512], F32, tag="av", name="av_a")
                av_b = avps.tile([VW, 512], F32, tag="av", name="av_b")
                for b in range(nb):
                    diag = (b // 4 == c)
                    off = 128 * (b - 4 * c) if diag else 0
                    bs = slice(128 * b, 128 * b + 128)
                    strip = strips.tile([128, 1024], F32, tag="strip",
                                        name="strip")
                    et = etpool.tile([128, 1024], BF16, tag="et", name="et")
                    nc.tensor.matmul(
                        strip[:, off:512],
                        kt[0:64, bs],
                        qt[0:64, off:512],
                        start=True, stop=True,
                    )
                    nc.tensor.matmul(
                        strip[:, 512 + off : 1024],
                        kt[64:128, bs],
                        qt[64:128, off:512],
                        start=True, stop=True,
                    )
                    if off == 0:
                        nc.scalar.activation(
                            et[:, 0:1024], strip[:, 0:1024], AF.Exp
                        )
                    else:
                        # one instr over both heads' valid regions:
                        # cols [off,512) and [512+off,1024)
                        w_ = 512 - off
                        src_ap = bass.AP(
                            tensor=strip.tensor,
                            offset=strip.offset + off,
                            ap=[list(strip.ap[0]), [512, 2], [1, w_]],
                        )
                        dst_ap = bass.AP(
                            tensor=et.tensor,
                            offset=et.offset + off,
                            ap=[list(et.ap[0]), [512, 2], [1, w_]],
                        )
                        nc.scalar.activation(dst_ap, src_ap, AF.Exp)
                    if diag:
                        for bcol in (off, 512 + off):
                            nc.gpsimd.affine_select(
                                out=et[:, bcol : bcol + 128],
                                in_=et[:, bcol : bcol + 128],
                                compare_op=mybir.AluOpType.is_ge,
                                fill=0.0,
                                base=0,
                                pattern=[[1, 128]],
                                channel_multiplier=-1,
                            )
                    nc.tensor.matmul(
                        av_a[:, off:512],
                        v_sb[b][:, VW * ha : VW * ha + VW],
                        et[:, off:512],
                        start=(b == 0), stop=(b == nb - 1),
                    )
                    nc.tensor.matmul(
                        av_b[:, off:512],
                        v_sb[b][:, VW * hb : VW * hb + VW],
                        et[:, 512 + off : 1024],
                        start=(b == 0), stop=(b == nb - 1),
                    )
                osb_t = opool.tile([128, 512], BF16, tag="osb", name="osb")
                for h, av in ((0, av_a), (1, av_b)):
                    # evict PSUM early (frees the bank for the next unit)
                    oraw = opsb.tile([64, 512], F32, tag="oraw", name="oraw")
                    nc.vector.tensor_copy(oraw, av[0:64, :])
                    den = opsb.tile([1, 512], F32, tag="den", name="den",
                                    bufs=2)
                    nc.vector.tensor_copy(den, av[64:65, :])
                    rec = opsb.tile([1, 512], F32, tag="rec", name="rec",
                                    bufs=2)
                    scr = opsb.tile([1, 512], F32, tag="scr", name="scr",
                                    bufs=2)
                    nc.vector.reciprocal_approx_accurate(
                        rec, den, scratch=scr
                    )
                    # broadcast 1/den across 64 partitions via a DRAM bounce
                    rd = dram.tile([1, 512], F32, tag="rd", name="rd")
                    nc.sync.dma_start(out=rd, in_=rec)
                    bc = opsb.tile([64, 512], F32, tag="bc", name="bc",
                                   bufs=2)
                    nc.sync.dma_start(
                        out=bc,
                        in_=bass.AP(tensor=rd.tensor, offset=rd.offset,
                                    ap=[[0, 64]] + list(rd.ap[1:])),
                    )
                    nc.vector.tensor_mul(
                        osb_t[64 * h : 64 * h + 64, :],
                        oraw[0:64, :],
                        bc,
                    )
                return osb_t

            def proj_chunk(c, osbs):
                for n in range(8):
                    acc = accps.tile([128, 512], F32, tag="acc", name="acc")
                    for k in range(4):
                        nc.tensor.matmul(
                            acc,
                            wp_sb[k][:, 128 * n : 128 * n + 128],
                            osbs[k],
                            start=(k == 0), stop=(k == 3),
                        )
                    yt = opsb.tile([128, 512], F32, tag="yt", name="yt",
                                   bufs=4)
                    nc.vector.tensor_copy(yt, acc)
                    nc.sync.dma_start(
                        out=yT[128 * n : 128 * n + 128,
                               512 * c : 512 * c + 512],
                        in_=yt,
                    )

            qts = [qpool.tile([128, 512], BF16, tag="qt", name="qt")
                   for _ in range(PAIRS)]
            qkv_v(0, xt0)
            qkv_qk(0, qts, xt0)
            for c in range(CH):
                if c + 1 < CH:
                    xt_next = load_xt(c + 1)
                    qts2 = [qpool.tile([128, 512], BF16, tag="qt", name="qt")
                            for _ in range(PAIRS)]
                with nc.named_scope(f"attn{c}"):
                    osbs = [attn_unit(0, c, qts[0]),
                            attn_unit(1, c, qts[1])]
                if c + 1 < CH:
                    with nc.named_scope(f"qkv{c + 1}"):
                        qkv_v(c + 1, xt_next)
                with nc.named_scope(f"attn{c}b"):
                    osbs += [attn_unit(2, c, qts[2]),
                             attn_unit(3, c, qts[3])]
                if c + 1 < CH:
                    with nc.named_scope(f"qkv{c + 1}"):
                        qkv_qk(c + 1, qts2, xt_next)
                    qts = qts2
                with nc.named_scope(f"proj{c}"):
                    proj_chunk(c, osbs)
    return nc


_prog = None


def _get_program():
    global _prog
    if _prog is None:
        _prog = build(bacc.Bacc(None))
        _prog.finalize()
    return _prog


def make_in_maps(x, w_qkv, b_qkv, w_proj):
    x = np.ascontiguousarray(np.asarray(x, np.float32))
    w_qkv = np.asarray(w_qkv, np.float32)
    b_qkv = np.asarray(b_qkv, np.float32)
    w_proj = np.asarray(w_proj, np.float32)
    in_maps = []
    for core in range(8):
        b, g = divmod(core, 2)
        gs = slice(DL * g, DL * g + DL)
        gk = slice(D + DL * g, D + DL * g + DL)
        gv = slice(2 * D + DL * g, 2 * D + DL * g + DL)
        in_maps.append({
            "xT": np.ascontiguousarray(x[b].T),
            "wq": np.ascontiguousarray(w_qkv[:, gs]) * np.float32(0.125),
            "wk": np.ascontiguousarray(w_qkv[:, gk]),
            "wv": np.ascontiguousarray(w_qkv[:, gv]),
            "bq": np.ascontiguousarray(b_qkv[gs]) * np.float32(0.125),
            "bk": np.ascontiguousarray(b_qkv[gk]),
            "bv": np.ascontiguousarray(b_qkv[gv]),
            "wp": np.ascontiguousarray(
                w_proj[DL * g : DL * g + DL, :]).astype(ml_dtypes.bfloat16),
            "ones8": np.ones((128, 16), ml_dtypes.bfloat16),
        })
    return in_maps


def combine_outputs(results, b_proj):
    b_proj = np.asarray(b_proj, np.float32)
    y = np.empty((B, T, D), np.float32)
    for b in range(B):
        yt = results[2 * b]["yT"] + results[2 * b + 1]["yT"]
        y[b] = yt.T + b_proj
    return y


def kernel(x, w_qkv, b_qkv, w_proj, b_proj, **run_kwargs):
    in_maps = make_in_maps(x, w_qkv, b_qkv, w_proj)
    r = run_bass_kernel_spmd(_get_program(), in_maps,
                             core_ids=list(range(8)), **run_kwargs)
    out = combine_outputs(r.results, b_proj)
    kernel.last_result = r
    return out


# revision 49
# speedup vs baseline: 1.0910x; 1.0142x over previous
"""Causal multi-head attention (B=4, T=2048, D=1024, H=16, HD=64) on 8
Trainium2 NeuronCores.

Sharding: data-parallel over batch (4) x tensor-parallel over heads (2
groups of 8). Each core runs the same Bass program on its own input
slices; the host sums the two tensor-parallel partial projections per
batch and adds b_proj.

Per-core dataflow (feature-major, no on-chip transposes), software-
pipelined across chunks of 512 query tokens so ScalarE exp overlaps the
QKV / proj matmuls:

  chunk c:  QKV(c) on PE  ->  attn units (4 head-pairs) for chunk c
            (S row-tiled 64-deep matmul pairs, exp on ScalarE, AV in
            bf16)  ->  proj(c), while QKV(c+1) fills PE gaps.

All weights persist in SBUF (loaded once). x and the qkv/proj weights
stream as float32r (1 cycle/row at N>=256); Q, K, V, exp(S), O and
w_proj are bf16 (1 cycle/row at every N + fast weight load for the
S/proj stationaries). Scores accumulate in f32 PSUM.
"""

import numpy as np
import ml_dtypes

import concourse.bass as bass
import concourse.bacc as bacc
import concourse.mybir as mybir
import concourse.tile as tile
from concourse.bass_utils import run_bass_kernel_spmd

F32 = mybir.dt.float32
F32R = mybir.dt.float32r
BF16 = mybir.dt.bfloat16
AF = mybir.ActivationFunctionType

B, T, D = 4, 2048, 1024
H, HD = 16, 64
NH = 8          # heads per core
DL = NH * HD    # 512 local qkv feature dim
PAIRS = NH // 2
CH = T // 512   # 4 chunks of 512 tokens
KT = T // 128   # 16 tk blocks / token tiles
VW = 66         # V columns per head incl. ones column + pad (4B alignment)


def build(nc: bass.Bass):
    xT = nc.declare_dram_parameter("xT", [D, T], F32R, isOutput=False)
    wq = nc.declare_dram_parameter("wq", [D, DL], F32R, isOutput=False)
    wk = nc.declare_dram_parameter("wk", [D, DL], F32R, isOutput=False)
    wv = nc.declare_dram_parameter("wv", [D, DL], F32R, isOutput=False)
    bq = nc.declare_dram_parameter("bq", [DL], F32, isOutput=False)
    bk = nc.declare_dram_parameter("bk", [DL], F32, isOutput=False)
    bv = nc.declare_dram_parameter("bv", [DL], F32, isOutput=False)
    wp = nc.declare_dram_parameter("wp", [DL, D], BF16, isOutput=False)
    ones8 = nc.declare_dram_parameter("ones8", [128, 16], BF16,
                                      isOutput=False)
    yT = nc.declare_dram_parameter("yT", [D, T], F32, isOutput=True)

    with tile.TileContext(nc) as tc:
        with (
            tc.tile_pool(name="persist", bufs=1) as persist,
            tc.tile_pool(name="qpool", bufs=8) as qpool,
            tc.tile_pool(name="opool", bufs=8) as opool,
            tc.tile_pool(name="xpool", bufs=16) as xpool,
            tc.tile_pool(name="etpool", bufs=4) as etpool,
            tc.tile_pool(name="opsb", bufs=4) as opsb,
            tc.tile_pool(name="accps", bufs=2, space="PSUM") as accps,
            tc.tile_pool(name="strips", bufs=2, space="PSUM") as strips,
            tc.tile_pool(name="avps", bufs=2, space="PSUM") as avps,
            tc.tile_pool(name="dram", bufs=4, space="DRAM") as dram,
        ):
            # -------- persistent tiles (weights, K^T, V) --------
            kT = [persist.tile([128, T], BF16, tag=f"kt{p}", name=f"kt{p}")
                  for p in range(PAIRS)]
            v_sb = [persist.tile([128, NH * VW], BF16, tag=f"v{i}",
                                 name=f"v{i}") for i in range(KT)]
            wq_sb = [persist.tile([128, DL], F32R, tag=f"wq{k}",
                                  name=f"wq{k}") for k in range(8)]
            wk_sb = [persist.tile([128, DL], F32R, tag=f"wk{k}",
                                  name=f"wk{k}") for k in range(8)]
            wv_sb = [persist.tile([128, DL], F32R, tag=f"wv{k}",
                                  name=f"wv{k}") for k in range(8)]
            wp_sb = [persist.tile([128, D], BF16, tag=f"wp{k}",
                                  name=f"wp{k}") for k in range(4)]
            bq_sb = persist.tile([128, 4], F32, tag="bq", name="bq_sb")
            bk_sb = persist.tile([128, 4], F32, tag="bk", name="bk_sb")
            bv_bc = persist.tile([128, DL], F32, tag="bv", name="bv_bc")
            def load_xt(c):
                cs = slice(512 * c, 512 * c + 512)
                xt = []
                for k in range(8):
                    t_ = xpool.tile([128, 512], F32R, tag="xt", name="xt")
                    nc.sync.dma_start(
                        out=t_, in_=xT[128 * k : 128 * k + 128, cs]
                    )
                    xt.append(t_)
                return xt

            # V-projection weights + x chunk 0 first: they gate the first
            # matmuls.  Everything else loads behind them.
            for k in range(8):
                nc.sync.dma_start(out=wv_sb[k],
                                  in_=wv[128 * k : 128 * k + 128, :])
            xt0 = load_xt(0)
            nc.sync.dma_start(
                out=bv_bc,
                in_=bass.AP(tensor=bv[:].tensor, offset=0,
                            ap=[[0, 128], [1, DL]]),
            )
            # ones (+pad) columns of every V tile, written once
            for tt in range(KT):
                vv = v_sb[tt].rearrange("p (h c) -> p h c", c=VW)
                nc.sync.dma_start(
                    out=vv[:, :, HD : HD + 2],
                    in_=ones8[:, 0:16].rearrange("p (h c) -> p h c", c=2),
                )
            for k in range(8):
                nc.sync.dma_start(out=wq_sb[k],
                                  in_=wq[128 * k : 128 * k + 128, :])
                nc.sync.dma_start(out=wk_sb[k],
                                  in_=wk[128 * k : 128 * k + 128, :])
            for k in range(4):
                nc.sync.dma_start(out=wp_sb[k],
                                  in_=wp[128 * k : 128 * k + 128, :])
            nc.sync.dma_start(
                out=bq_sb, in_=bq[:].rearrange("(a p) -> p a", p=128)
            )
            nc.sync.dma_start(
                out=bk_sb, in_=bk[:].rearrange("(a p) -> p a", p=128)
            )

            def qkv_v(c, xt):
                """V projection for token chunk c -> v_sb[4c..4c+4]."""
                for t4 in range(4):
                    tt = 4 * c + t4
                    acc = accps.tile([128, 512], F32, tag="acc", name="acc")
                    for k in range(8):
                        nc.tensor.matmul(
                            acc,
                            xt[k][:, 128 * t4 : 128 * t4 + 128],
                            wv_sb[k],
                            start=(k == 0), stop=(k == 7),
                        )
                    for h in range(NH):
                        nc.vector.tensor_add(
                            v_sb[tt][:, VW * h : VW * h + HD],
                            acc[:, HD * h : HD * h + HD],
                            bv_bc[:, HD * h : HD * h + HD],
                        )

            def qkv_qk(c, qts, xt):
                """Q^T / K^T projections for token chunk c."""
                cs = slice(512 * c, 512 * c + 512)
                for w_sb, b_sb, dsts in (
                    (wq_sb, bq_sb, [(qts[n], slice(0, 512)) for n in range(4)]),
                    (wk_sb, bk_sb, [(kT[n], cs) for n in range(4)]),
                ):
                    for n in range(4):
                        acc = accps.tile([128, 512], F32, tag="acc",
                                         name="acc")
                        for k in range(8):
                            nc.tensor.matmul(
                                acc,
                                w_sb[k][:, 128 * n : 128 * n + 128],
                                xt[k],
                                start=(k == 0), stop=(k == 7),
                            )
                        dst, dcs = dsts[n]
                        nc.vector.tensor_scalar_add(
                            out=dst[:, dcs],
                            in0=acc,
                            scalar1=b_sb[:, n : n + 1],
                        )

            def attn_unit(p, c, qt):
                """Attention for head-pair p, query chunk c. Returns the
                normalized O^T tile [128, 512] (bf16)."""
                kt = kT[p]
                ha, hb = 2 * p, 2 * p + 1
                nb = 4 * (c + 1)
                av_a = avps.tile([VW, 512], F32, tag="av", name="av_a")
                av_b = avps.tile([VW, 512], F32, tag="av", name="av_b")
                for b in range(nb):
                    diag = (b // 4 == c)
                    off = 128 * (b - 4 * c) if diag else 0
                    bs = slice(128 * b, 128 * b + 128)
                    strip = strips.tile([128, 1024], F32, tag="strip",
                                        name="strip")
                    et = etpool.tile([128, 1024], BF16, tag="et", name="et")
                    nc.tensor.matmul(
                        strip[:, off:512],
                        kt[0:64, bs],
                        qt[0:64, off:512],
                        start=True, stop=True,
                    )
                    nc.tensor.matmul(
                        strip[:, 512 + off : 1024],
                        kt[64:128, bs],
                        qt[64:128, off:512],
                        start=True, stop=True,
                    )
                    if off == 0:
                        nc.scalar.activation(
                            et[:, 0:1024], strip[:, 0:1024], AF.Exp
                        )
                    else:
                        # one instr over both heads' valid regions:
                        # cols [off,512) and [512+off,1024)
                        w_ = 512 - off
                        src_ap = bass.AP(
                            tensor=strip.tensor,
                            offset=strip.offset + off,
                            ap=[list(strip.ap[0]), [512, 2], [1, w_]],
                        )
                        dst_ap = bass.AP(
                            tensor=et.tensor,
                            offset=et.offset + off,
                            ap=[list(et.ap[0]), [512, 2], [1, w_]],
                        )
                        nc.scalar.activation(dst_ap, src_ap, AF.Exp)
                    if diag:
                        for bcol in (off, 512 + off):
                            nc.gpsimd.affine_select(
                                out=et[:, bcol : bcol + 128],
                                in_=et[:, bcol : bcol + 128],
                                compare_op=mybir.AluOpType.is_ge,
                                fill=0.0,
                                base=0,
                                pattern=[[1, 128]],
                                channel_multiplier=-1,
                            )
                    nc.tensor.matmul(
                        av_a[:, off:512],
                        v_sb[b][:, VW * ha : VW * ha + VW],
                        et[:, off:512],
                        start=(b == 0), stop=(b == nb - 1),
                    )
                    nc.tensor.matmul(
                        av_b[:, off:512],
                        v_sb[b][:, VW * hb : VW * hb + VW],
                        et[:, 512 + off : 1024],
                        start=(b == 0), stop=(b == nb - 1),
                    )
                osb_t = opool.tile([128, 512], BF16, tag="osb", name="osb")
                for h, av in ((0, av_a), (1, av_b)):
                    # evict PSUM early (frees the bank for the next unit)
                    oraw = opsb.tile([64, 512], F32, tag="oraw", name="oraw")
                    nc.vector.tensor_copy(oraw, av[0:64, :])
                    den = opsb.tile([1, 512], F32, tag="den", name="den",
                                    bufs=2)
                    nc.vector.tensor_copy(den, av[64:65, :])
                    rec = opsb.tile([1, 512], F32, tag="rec", name="rec",
                                    bufs=2)
                    scr = opsb.tile([1, 512], F32, tag="scr", name="scr",
                                    bufs=2)
                    nc.vector.reciprocal_approx_accurate(
                        rec, den, scratch=scr
                    )
                    # broadcast 1/den across 64 partitions via a DRAM bounce
                    rd = dram.tile([1, 512], F32, tag="rd", name="rd")
                    nc.sync.dma_start(out=rd, in_=rec)
                    bc = opsb.tile([64, 512], F32, tag="bc", name="bc",
                                   bufs=2)
                    nc.sync.dma_start(
                        out=bc,
                        in_=bass.AP(tensor=rd.tensor, offset=rd.offset,
                                    ap=[[0, 64]] + list(rd.ap[1:])),
                    )
                    nc.vector.tensor_mul(
                        osb_t[64 * h : 64 * h + 64, :],
                        oraw[0:64, :],
                        bc,
                    )
                return osb_t

            def proj_chunk(c, osbs):
                for n in range(8):
                    acc = accps.tile([128, 512], F32, tag="acc", name="acc")
                    for k in range(4):
                        nc.tensor.matmul(
                            acc,
                            wp_sb[k][:, 128 * n : 128 * n + 128],
                            osbs[k],
                            start=(k == 0), stop=(k == 3),
                        )
                    yt = opsb.tile([128, 512], F32, tag="yt", name="yt",
                                   bufs=4)
                    nc.vector.tensor_copy(yt, acc)
                    nc.sync.dma_start(
                        out=yT[128 * n : 128 * n + 128,
                               512 * c : 512 * c + 512],
                        in_=yt,
                    )

            qts = [qpool.tile([128, 512], BF16, tag="qt", name="qt")
                   for _ in range(PAIRS)]
            qkv_v(0, xt0)
            qkv_qk(0, qts, xt0)
            for c in range(CH):
                if c + 1 < CH:
                    xt_next = load_xt(c + 1)
                    qts2 = [qpool.tile([128, 512], BF16, tag="qt", name="qt")
                            for _ in range(PAIRS)]
                with nc.named_scope(f"attn{c}"):
                    osbs = [attn_unit(0, c, qts[0]),
                            attn_unit(1, c, qts[1])]
                if c + 1 < CH:
                    with nc.named_scope(f"qkv{c + 1}"):
                        qkv_v(c + 1, xt_next)
                with nc.named_scope(f"attn{c}b"):
                    osbs += [attn_unit(2, c, qts[2]),
                             attn_unit(3, c, qts[3])]
                if c + 1 < CH:
                    with nc.named_scope(f"qkv{c + 1}"):
                        qkv_qk(c + 1, qts2, xt_next)
                    qts = qts2
                with nc.named_scope(f"proj{c}"):
                    proj_chunk(c, osbs)
    return nc


_prog = None


def _get_program():
    global _prog
    if _prog is None:
        _prog = build(bacc.Bacc(None))
        _prog.finalize()
    return _prog


def make_in_maps(x, w_qkv, b_qkv, w_proj):
    x = np.ascontiguousarray(np.asarray(x, np.float32))
    w_qkv = np.asarray(w_qkv, np.float32)
    b_qkv = np.asarray(b_qkv, np.float32)
    w_proj = np.asarray(w_proj, np.float32)
    in_maps = []
    for core in range(8):
        b, g = divmod(core, 2)
        gs = slice(DL * g, DL * g + DL)
        gk = slice(D + DL * g, D + DL * g + DL)
        gv = slice(2 * D + DL * g, 2 * D + DL * g + DL)
        in_maps.append({
            "xT": np.ascontiguousarray(x[b].T),
            "wq": np.ascontiguousarray(w_qkv[:, gs]) * np.float32(0.125),
            "wk": np.ascontiguousarray(w_qkv[:, gk]),
            "wv": np.ascontiguousarray(w_qkv[:, gv]),
            "bq": np.ascontiguousarray(b_qkv[gs]) * np.float32(0.125),
            "bk": np.ascontiguousarray(b_qkv[gk]),
            "bv": np.ascontiguousarray(b_qkv[gv]),
            "wp": np.ascontiguousarray(
                w_proj[DL * g : DL * g + DL, :]).astype(ml_dtypes.bfloat16),
            "ones8": np.ones((128, 16), ml_dtypes.bfloat16),
        })
    return in_maps


def combine_outputs(results, b_proj):
    b_proj = np.asarray(b_proj, np.float32)
    y = np.empty((B, T, D), np.float32)
    for b in range(B):
        yt = results[2 * b]["yT"] + results[2 * b + 1]["yT"]
        y[b] = yt.T + b_proj
    return y


def kernel(x, w_qkv, b_qkv, w_proj, b_proj, **run_kwargs):
    in_maps = make_in_maps(x, w_qkv, b_qkv, w_proj)
    r = run_bass_kernel_spmd(_get_program(), in_maps,
                             core_ids=list(range(8)), **run_kwargs)
    out = combine_outputs(r.results, b_proj)
    kernel.last_result = r
    return out
